# revision 68
# baseline (speedup 1.0000x reference)
"""Trainium2 Bass kernel for PixelUnshuffle->MHA->PixelShuffle (nn_Attention).

Reference computation (per batch element, 8 batch elements data-parallel
across 8 NeuronCores):
  x [64, 256, 256] --PixelUnshuffle(8)--> tokens [N=1024, C=4096]
  qkv = tokens @ W_qkv            [1024, 768]
  4-head attention (d=64), softmax over tokens
  y = attn_out @ W_out + b_out    [1024, 4096]
  --PixelShuffle(8)--> [64, 256, 256]

v2 structure (vs the v1 three-phase kernel):
  * ALL HBM loads (x, W_qkv, W_out, bias) are issued as a handful of
    large descriptors at t=0 so the PE never waits on DMA and the HAM
    clock never drops out of the k8 p-state mid-kernel.
  * Stages 2+3 are fused into a 4-chunk pipeline over 256-token query
    chunks: attention for chunk c overlaps the output projection, PSUM
    evacuation and DRAM writes of chunk c-1, spreading the 16 MB of f32
    output writes across the whole second half of the kernel.
  * Heads 2,3 use a 96-column padded v block (ones at col 31, d at cols
    32..95, matmul output base partition 32) so their attention output
    lands directly on partitions 64..127 -- no partition-shift DMA.

Layout notes (identical to v1 where not mentioned):
  Token index   n = hh*32 + ww            (hh, ww in [0,32))
  Channel index c = c0*64 + r1*8 + r2     (c0 in [0,64), r1, r2 in [0,8))
  x[c0, hh*8+r1, ww*8+r2] = tokens[n, c]
  Attention is computed transposed (dotsT[m, n]) so no transposes are
  needed anywhere; softmax denominators ride along as an extra ones
  column in v (row 64 / row 63 of the augmented output) and are
  broadcast across partitions with a 0-stride DRAM read.
"""

import sys

if "/opt/trn_rl_repo" not in sys.path:
    sys.path.insert(0, "/opt/trn_rl_repo")

import os

import ml_dtypes
import numpy as np

import concourse.bass as bass
from concourse import bacc, mybir, tile
from concourse.bass_utils import run_bass_kernel_spmd

F32 = mybir.dt.float32
BF16 = mybir.dt.bfloat16

SCALE = 0.125  # DIM_HEAD ** -0.5

_CACHE = {}


def _build(zero_bias=False, debug_outs=False):
    nc = bacc.Bacc("TRN2", target_bir_lowering=False, debug=False, num_devices=8)

    x_d = nc.dram_tensor("x", [64, 256, 256], BF16, kind="ExternalInput").ap()
    wq_d = nc.dram_tensor("W_qkv", [4096, 768], BF16, kind="ExternalInput").ap()
    wo_d = nc.dram_tensor("W_out", [256, 4096], BF16, kind="ExternalInput").ap()
    b_d = nc.dram_tensor("b_out", [4096], F32, kind="ExternalInput").ap()
    out_d = nc.dram_tensor("out", [64, 256, 256], F32, kind="ExternalOutput").ap()

    zrc_d = nc.dram_tensor("zr_scratch", [4, 1024], F32).ap()

    dbg = None
    if debug_outs:
        dbg = {
            "qkT": nc.dram_tensor(
                "dbg_qkT", [128, 4, 1024], BF16, kind="ExternalOutput"
            ).ap(),
            "v_sb": nc.dram_tensor(
                "dbg_v", [128, 8, 4, 128], BF16, kind="ExternalOutput"
            ).ap(),
            "outT": nc.dram_tensor(
                "dbg_outT", [128, 2, 1024], BF16, kind="ExternalOutput"
            ).ap(),
        }

    def dram_ap(base, off, pattern):
        return bass.AP(tensor=base.tensor, offset=base.offset + off, ap=pattern)

    with tile.TileContext(nc) as tc:
        _build_tiled(
            nc, tc, x_d, wq_d, wo_d, b_d, out_d, zrc_d, dram_ap, zero_bias, dbg
        )
    nc.compile()
    return nc


def _build_tiled(nc, tc, x_d, wq_d, wo_d, b_d, out_d, zrc_d, dram_ap, zero_bias, dbg=None):
    from contextlib import ExitStack

    with ExitStack() as ctx:
        pers = ctx.enter_context(tc.tile_pool(name="pers", bufs=1))

        # ---- persistent tiles ----
        # qkT[d-part, ot, n] : ot 0,1 = q dims 0..128,128..256; ot 2,3 = k
        qkT = pers.tile([128, 4, 1024], BF16)
        # v_sb[m-part, mc, h, 128] bf16.
        #   h even (h2=0): col 0..63 = v_d, col 64 = ones (rest zero)
        #   h odd  (h2=1): col 32 = ones, cols 64..127 = v_d (rest zero)
        # so the attn@v output for odd heads lands on partitions 64..127
        # directly (Z on partition 32, which keeps engine reads of the Z
        # row 32-partition-aligned) -- no partition-shift DMA needed.
        v_sb = pers.tile([128, 8, 4, 128], BF16)
        # outT[i-part, ic, n] : i = h*64+d ; ic = i//128 (normalized attn out)
        outT = pers.tile([128, 2, 1024], BF16)
        # bias[c-part, r2, cg]
        bias_sb = pers.tile([128, 8, 4], F32)
        # W_out [i-part, ic, c_perm]
        wo_sb = pers.tile([128, 2, 4096], BF16)

        nc.vector.memset(v_sb[:], 0.0)
        nc.vector.memset(v_sb[:, :, 0:4:2, 64:65], 1.0)
        nc.vector.memset(v_sb[:, :, 1:4:2, 32:33], 1.0)

        # =========================== stage 1 ===========================
        # QKV projection with pixel-unshuffle folded in. 2 windows of 512
        # tokens (hh-halves).
        with (
            tc.tile_pool(name="wq", bufs=1) as wqp,
            tc.tile_pool(name="xw", bufs=1) as xwp,
            tc.tile_pool(name="ps1", bufs=1, space="PSUM") as ps1,
        ):
            wq_sb = wqp.tile([128, 8, 4, 768], BF16)  # [c-part, r2, cg, o]
            # x staging, natural DMA layout: [c-part, (w,cg), hh, (ww,r2)]
            xt_all = wqp.tile([128, 8, 16, 256], BF16)

            # ---------- prefetch: issue EVERY load up front ----------
            # All x loads are per-hh descriptors (4KB contiguous runs ->
            # full per-queue bandwidth; 512B-run bulk descriptors measured
            # only ~35 GB/s/queue). wq (6.3MB) is split across gpsimd
            # (r2 0-3) and scalar (r2 4-7, slotted behind the first two
            # h1 x half-loads) since one queue at ~170GB/s cannot deliver
            # it before the stage-1 r2 sweep consumes it.
            def load_x_hh(eng, w, cg, hh):
                eng.dma_start(
                    out=xt_all[:, w * 4 + cg, hh, :],
                    in_=dram_ap(
                        x_d,
                        cg * 16 * 65536 + (w * 16 + hh) * 2048,
                        [[65536, 16], [1, 2048]],
                    ),
                )

            def load_wq(eng, r2):
                eng.dma_start(
                    out=wq_sb[:, r2, :, :],
                    in_=dram_ap(
                        wq_d,
                        r2 * 512 * 768,
                        [[768, 128], [98304, 4], [1, 768]],
                    ),
                )

            nc.gpsimd.dma_start(
                out=bias_sb[:],
                in_=dram_ap(b_d, 0, [[32, 128], [4, 8], [1, 4]]),
            )
            for r2 in range(4):
                load_wq(nc.gpsimd, r2)
            # scalar: h1 halves of w0 cg0/cg1, then the late wq r2s
            for cg in range(2):
                for hh in range(8, 16):
                    load_x_hh(nc.scalar, 0, cg, hh)
            for r2 in range(4, 8):
                load_wq(nc.scalar, r2)
            # sync: h0 halves of w0 cg0/cg1, then cg2/cg3 in full
            for cg in range(2):
                for hh in range(8):
                    load_x_hh(nc.sync, 0, cg, hh)
            for cg in range(2, 4):
                for hh in range(16):
                    load_x_hh(nc.sync, 0, cg, hh)
            # gpsimd: window-1 x behind its wq share, then W_out (only
            # needed by the projection stage)
            for cg in range(4):
                for hh in range(16):
                    load_x_hh(nc.gpsimd, 1, cg, hh)
            nc.gpsimd.dma_start(
                out=wo_sb[:],
                in_=dram_ap(wo_d, 0, [[4096, 128], [524288, 2], [1, 4096]]),
            )

            # PE warmup: dummy matmuls so HAM ramps toward 2.4 GHz while
            # the prefetch DMAs land.
            warm = wqp.tile([128, 512], BF16)
            nc.vector.memset(warm[:], 0.0)
            warm_ps = ps1.tile([128, 512], F32, tag="qk0", bufs=1)
            for i in range(16):
                nc.tensor.matmul(
                    warm_ps[:], warm[:, 0:128], warm[:], start=True, stop=True
                )

            def make_xtb(w, cg):
                # xtb[c-part, r2, hh(16), ww] bf16, de-strided from xt_all
                xtb = xwp.tile([128, 8, 16, 32], BF16, tag="xtb", bufs=4)
                for half in range(2):
                    # (hh, ww, r2) -> (r2, hh, ww)
                    src = xt_all[:, w * 4 + cg, half * 8 : (half + 1) * 8, :]
                    src = src.rearrange("p hh (ww r2) -> p hh ww r2", r2=8)
                    dst = xtb[:, :, half * 8 : (half + 1) * 8, :]
                    csrc = src.transpose([0, 3, 1, 2])
                    nc.vector.tensor_copy(dst, csrc)
                return xtb

            for w in range(2):
                qks = [
                    ps1.tile([128, 512], F32, tag=f"qk{ot}", bufs=1, name=f"qk_{w}_{ot}")
                    for ot in range(4)
                ]
                vps = [
                    ps1.tile([128, 256], F32, tag=f"v{s}", bufs=1, name=f"v_{w}_{s}")
                    for s in range(4)
                ]
                # w0 visits its (cg, r2) accumulation cells in wq-arrival
                # order: the late wq r2s (4-7, on the scalar queue) are
                # only consumed after cg0/cg1's early-r2 cells.
                if w == 0:
                    cells = (
                        [(0, r) for r in range(4)]
                        + [(1, r) for r in range(4)]
                        + [(0, r) for r in range(4, 8)]
                        + [(1, r) for r in range(4, 8)]
                        + [(2, r) for r in range(8)]
                        + [(3, r) for r in range(8)]
                    )
                else:
                    cells = [(cg, r) for cg in range(4) for r in range(8)]
                xtb_by_cg = {}
                for ci, (cg, r2) in enumerate(cells):
                    if cg not in xtb_by_cg:
                        xtb_by_cg[cg] = make_xtb(w, cg)
                    xtb = xtb_by_cg[cg]
                    first = ci == 0
                    last = ci == 31
                    for ot in range(4):
                        nc.tensor.matmul(
                            qks[ot][:],
                            wq_sb[:, r2, cg, ot * 128 : (ot + 1) * 128],
                            xtb[:, r2, :, :],
                            start=first,
                            stop=last,
                        )
                    for s in range(4):
                        nc.tensor.matmul(
                            vps[s][:],
                            xtb[:, r2, 4 * s : 4 * s + 4, :],
                            wq_sb[:, r2, cg, 512:768],
                            start=first,
                            stop=last,
                        )
                for ot in range(4):
                    dst = qkT[:, ot, w * 512 : (w + 1) * 512]
                    if ot % 2 == 0:
                        nc.scalar.copy(dst, qks[ot][:])
                    else:
                        nc.vector.tensor_copy(dst, qks[ot][:])
                for s in range(4):
                    mc = 4 * w + s
                    # even heads -> cols 0:64 ; odd heads -> cols 64:128
                    vh = vps[s][:].rearrange("p (h d) -> p h d", h=4)
                    nc.vector.tensor_copy(v_sb[:, mc, 0:4:2, 0:64], vh[:, 0:4:2, :])
                    nc.scalar.copy(v_sb[:, mc, 1:4:2, 64:128], vh[:, 1:4:2, :])

        if dbg is not None:
            nc.gpsimd.dma_start(out=dbg["qkT"][:], in_=qkT[:])
            nc.gpsimd.dma_start(out=dbg["v_sb"][:], in_=v_sb[:])

        bisect = int(os.environ.get("K_BISECT", "0"))
        if bisect == 1:
            return

        # ================== stage 2+3: fused attention + proj ==========
        # 4 chunks of 256 query tokens. Per chunk: attention for all 4
        # heads -> z reciprocal broadcast -> normalized outT -> output
        # projection + pixel-shuffle evacuation + DRAM writes, which all
        # overlap the next chunk's attention.
        with (
            tc.tile_pool(name="s2", bufs=1) as s2,
            tc.tile_pool(name="psA", bufs=1, space="PSUM") as psA,
        ):
            def attn_window(w):
                # attention over a full 512-token window: dots/exp/attn@v
                # all run at N=512 (both halves of the 2-bank dt tile),
                # halving instruction counts vs 256-chunks.
                n0 = w * 512
                ztmp = s2.tile([65, 2, 512], F32, tag="ztmp", bufs=4, name=f"zt_{w}")
                for hp in range(2):
                    # the two h2 accumulation chains stay pending across
                    # the whole mc loop -> they MUST live in different
                    # PSUM banks (one full-bank tile each)
                    oa_lo = psA.tile(
                        [128, 512], F32, tag="oalo", bufs=1, name=f"oal_{w}_{hp}"
                    )
                    oa_hi = psA.tile(
                        [128, 512], F32, tag="oahi", bufs=1, name=f"oah_{w}_{hp}"
                    )
                    ed = s2.tile(
                        [128, 8, 1024], BF16, tag="ed", bufs=2, name=f"ed_{w}_{hp}"
                    )
                    for mc in range(8):
                        # the two h2 dots run CONCURRENTLY in disjoint PE
                        # row groups -> their outputs must land in
                        # different PSUM banks (HW faults otherwise)
                        dt = psA.tile(
                            [128, 2, 512], F32, tag="dt", bufs=2,
                            name=f"dt_{w}_{hp}_{mc}",
                        )
                        for h2 in range(2):
                            b = h2 * 64
                            nc.tensor.matmul(
                                dt[:, h2, :],
                                qkT[b : b + 64, 2 + hp, mc * 128 : (mc + 1) * 128],
                                qkT[b : b + 64, hp, n0 : n0 + 512],
                                start=True,
                                stop=True,
                            )
                        nc.scalar.activation(
                            ed[:, mc, :].rearrange("p (a b) -> p a b", a=2),
                            dt[:],
                            mybir.ActivationFunctionType.Exp,
                            scale=SCALE,
                        )
                        nc.tensor.matmul(
                            oa_lo[0:68, :],
                            v_sb[:, mc, 2 * hp, 0:68],
                            ed[:, mc, 0:512],
                            start=(mc == 0),
                            stop=(mc == 7),
                        )
                        nc.tensor.matmul(
                            oa_hi[:],
                            v_sb[:, mc, 2 * hp + 1, :],
                            ed[:, mc, 512:1024],
                            start=(mc == 0),
                            stop=(mc == 7),
                        )
                    # evacuate unnormalized attn out (scale-free in bf16)
                    # and the two z rows; this releases the oaug banks.
                    osb = s2.tile(
                        [128, 512], BF16, tag="osb", bufs=4, name=f"osb_{w}_{hp}"
                    )
                    nc.vector.tensor_copy(osb[0:64, :], oa_lo[0:64, :])
                    nc.vector.tensor_copy(osb[64:128, :], oa_hi[64:128, :])
                    nc.vector.tensor_copy(ztmp[64:65, hp, :], oa_lo[64:65, :])
                    nc.vector.tensor_copy(ztmp[32:33, hp, :], oa_hi[32:33, :])
                    _OSB[(w, hp)] = osb

                # ---- z chain for the whole window (4 z-rows of 512) ----
                # z16 flat order: row32 (h2=1: hp0,hp1) then row64 (h2=0)
                z16 = s2.tile([64, 32], F32, tag="z16", bufs=4, name=f"z16_{w}")
                nc.sync.dma_start(out=z16[0:32, :], in_=ztmp[32:33, :, :])
                nc.sync.dma_start(out=z16[32:64, :], in_=ztmp[64:65, :, :])
                z16r = s2.tile([64, 32], F32, tag="z16r", bufs=4, name=f"z16r_{w}")
                nc.vector.reciprocal(z16r[:], z16[:])
                nc.sync.dma_start(
                    out=dram_ap(zrc_d, w * 2048, [[32, 64], [1, 32]]),
                    in_=z16r[:],
                )
                # readback with 0-stride partition broadcast:
                # zrc[w*2048 +] [0:1024]=1/Z(h2=1: hp0,hp1) [1024:2048]=h2=0
                for hp in range(2):
                    zbc = s2.tile(
                        [128, 512], F32, tag="zbc", bufs=8, name=f"zbc_{w}_{hp}"
                    )
                    nc.sync.dma_start(
                        out=zbc[0:64, :],
                        in_=dram_ap(
                            zrc_d, w * 2048 + 1024 + hp * 512, [[0, 64], [1, 512]]
                        ),
                    )
                    nc.sync.dma_start(
                        out=zbc[64:128, :],
                        in_=dram_ap(zrc_d, w * 2048 + hp * 512, [[0, 64], [1, 512]]),
                    )
                    nc.vector.tensor_mul(
                        outT[:, hp, n0 : n0 + 512], _OSB[(w, hp)][:], zbc[:]
                    )

            def proj_window(w):
                # output projection over a full 512-token window: N=512
                # matmuls (one PSUM bank per c-tile), halving matmul and
                # evacuation instruction counts vs 256-token chunks.
                n0 = w * 512
                for ct in range(4):
                    y_t = s2.tile(
                        [128, 16, 32, 8], F32, tag="yt", bufs=4, name=f"yt_{w}_{ct}"
                    )
                    for r2 in range(8):
                        yb = psA.tile(
                            [128, 512], F32, tag="ybig", bufs=2,
                            name=f"yb_{w}_{ct}_{r2}",
                        )
                        for ic in range(2):
                            nc.tensor.matmul(
                                yb[:],
                                wo_sb[
                                    :,
                                    ic,
                                    r2 * 512
                                    + ct * 128 : r2 * 512
                                    + (ct + 1) * 128,
                                ],
                                outT[:, ic, n0 : n0 + 512],
                                start=(ic == 0),
                                stop=(ic == 1),
                            )
                        # evacuate with pixel-shuffle destride: src (hh,
                        # ww) -> slot r2 of y_t's (ww, r2) minor dims
                        dst = y_t[:, :, :, r2]
                        esrc = yb[:].rearrange("p (a b) -> p a b", a=16)
                        idx = ct * 8 + r2
                        if zero_bias:
                            # 5/8 vector, 3/8 scalar (scalar also runs exp)
                            if idx % 8 in (2, 5, 7):
                                nc.scalar.copy(dst, esrc)
                            else:
                                nc.vector.tensor_copy(dst, esrc)
                        else:
                            bias_bc = bias_sb[:, r2, ct][
                                :, None, None
                            ].broadcast_to([128, 16, 32])
                            nc.vector.tensor_add(dst, esrc, bias_bc)
                    # per-hh write descriptors: 128KB each, 8KB runs
                    for hh in range(16):
                        eng = (nc.sync, nc.gpsimd, nc.sync, nc.gpsimd,
                               nc.sync, nc.gpsimd, nc.gpsimd, nc.sync)[hh % 8]
                        eng.dma_start(
                            out=dram_ap(
                                out_d,
                                ct * 16 * 65536 + (w * 16 + hh) * 2048,
                                [[65536, 16], [1, 2048]],
                            ),
                            in_=y_t[:, hh, :, :],
                        )

            _OSB = {}
            attn_window(0)
            attn_window(1)
            if bisect != 2:
                proj_window(0)
                proj_window(1)
            if dbg is not None:
                nc.gpsimd.dma_start(out=dbg["outT"][:], in_=outT[:])


def _get_nc(zero_bias=False):
    key = f"nc_zb{int(zero_bias)}"
    if key not in _CACHE:
        _CACHE[key] = _build(zero_bias=zero_bias)
    return _CACHE[key]


def _prep_weights(W_qkv, W_out, b_out):
    # stage-1 contraction rows ordered [r2, c0, r1] (c0-major within each
    # 128-row block, matching the per-hh x staging partition layout)
    wq_perm = np.ascontiguousarray(
        W_qkv.reshape(64, 8, 8, 768).transpose(2, 0, 1, 3).reshape(4096, 768)
    ).astype(ml_dtypes.bfloat16)
    wo_perm = np.ascontiguousarray(
        W_out.reshape(256, 64, 8, 8).transpose(0, 3, 1, 2).reshape(256, 4096)
    ).astype(ml_dtypes.bfloat16)
    # b_perm[r2*512 + c0*8 + r1] = b_out[c0*64 + r1*8 + r2], then laid out
    # [p, r2, cg] where p = (c0 % 16)*8 + r1, cg = c0 // 16
    b_perm = b_out.reshape(64, 8, 8).transpose(2, 0, 1).reshape(4096)
    b_perm = np.ascontiguousarray(
        b_perm.reshape(8, 4, 128).transpose(2, 0, 1).reshape(4096)
    ).astype(np.float32)
    return wq_perm, wo_perm, b_perm


def kernel(x, W_qkv, W_out, b_out):
    nc = _get_nc(zero_bias=not np.any(np.asarray(b_out)))
    wq_perm, wo_perm, b_perm = _prep_weights(
        np.asarray(W_qkv, dtype=np.float32),
        np.asarray(W_out, dtype=np.float32),
        np.asarray(b_out, dtype=np.float32),
    )

    in_maps = [
        {
            "x": np.ascontiguousarray(x[b]).astype(ml_dtypes.bfloat16),
            "W_qkv": wq_perm,
            "W_out": wo_perm,
            "b_out": b_perm,
        }
        for b in range(8)
    ]
    trace = bool(int(os.environ.get("BENCH_TRACE", "0")))
    if trace:
        try:  # tracing needs the NTFF hook shim (see test.py); degrade if absent
            from antenv.axon_hooks import get_axon_ntff_profile_hook  # noqa: F401
        except ImportError:
            trace = False
    res = run_bass_kernel_spmd(nc, in_maps, core_ids=list(range(8)), trace=trace)
    if trace:
        _CACHE["last_result"] = res
    return np.stack([res.results[b]["out"] for b in range(8)]).astype(np.float32)
